# revision 22
# baseline (speedup 1.0000x reference)
"""Trainium2 Bass kernel for EquivariantBinaryClassificationNoGraphScalar.

Computation (see reference):
    s[b, c]  = sum_n x[b, n, c]                      # node-sum, N=256
    h        = LayerNorm_C(s) * ln_w + ln_b          # over C=1024
    out[b]   = sigmoid(h . W[0] + b)                 # Linear(C, 1)

Sharding: data-parallel over batch. x is [1024, 256, 1024] f32 (1 GiB);
each of 8 cores gets a [128, 256, 1024] shard (128 MiB) -> memory-bound,
per-core HBM roofline ~128MiB / 358GB/s ~= 375 us.

Per-core algorithm (v2 — node-sum on the TENSOR engine):
  The v1 kernel accumulated s[b, c] with 256 chained DVE tensor_adds;
  fp32 tensor_tensor runs 1x on DVE and every op pays a pipeline-DRAIN,
  so the DVE chain (~2.2us per [128,1024] add) was the ~460-610us
  bottleneck, not DMA.

  v2 streams x with partition = NODE instead of batch:
    tile[p, j, q, c] = x[b0+j, 2p + q, c]   (8 KiB contiguous/partition)
  and contracts the node axis on the PE: for batch b, a one-hot
  stationary (ones in column b, zeros elsewhere) routes
  sum_p x[b, 2p+q, c] (accumulated over q=0,1) into PSUM partition b:
    psum[m, c] += sum_p onehot_b[p, m] * tile[p, ., c]
  All 128 batches / node-halves accumulate into one [128, 1024] f32
  PSUM region (2 banks, fp32 out <=512 per matmul -> 2 c-chunks).
  Moving operand is bitcast to float32r: full-precision fp32 data path
  on the PE at 1 cycle/row for free-dim >= 256 (~0.2-0.25us per matmul,
  512 matmuls ~= 110-230us, hidden under the ~375us DMA stream).
  One-hot stationaries come from a sliding window over a [128, 257]
  zeros-except-column-128 SBUF strip: G[:, 128-b : 256-b].

  Epilogue for all 128 batches at once (s in SBUF, partition = batch):
  bn_stats/bn_aggr -> mu, var; rstd = 1/sqrt(var+eps);
  logits = rstd*(s.wln - mu*sum(wln)) + c0 with wln = ln_w*W[0],
  c0 = sum(ln_b*W[0]) + b; sigmoid on ScalarE.
"""

import sys

import numpy as np

if "/opt/trn_rl_repo" not in sys.path:
    sys.path.insert(0, "/opt/trn_rl_repo")

from contextlib import ExitStack

import concourse.bacc as bacc
import concourse.bass as bass
import concourse.tile as tile
from concourse import mybir
from concourse.bass_utils import run_bass_kernel_spmd

B, N, C = 1024, 256, 1024
NCORES = 8
BS = B // NCORES  # 128 batches per core
P = 128
NH = N // P  # node halves
FP32 = mybir.dt.float32
FP32R = mybir.dt.float32r
LN_EPS = 1e-5

BPD = 2  # batches per x DMA -> 2 MiB per transfer
X_BUFS = 6
MM_FD = 512  # moving free-dim per matmul (one fp32 PSUM bank)
DMA_RINGS = 2  # alternate x DMAs across qSPDynamicHW / qActDynamicHW

# Kept for test.py: the BassKernelResults of the last kernel() call
# (exec_time_ns is populated when BASS_TRACE=1).
LAST_RESULT = None


def build(bs: int = BS, bpd: int = BPD, x_bufs: int = X_BUFS, passes: int = 1):
    """Build the per-core Bass module. bs<128 gives a small variant for sim.

    passes>1 streams x that many times (PSUM accumulation restarts each
    pass via start=True; result unchanged) — used by test.py to measure
    pure device time per pass as slope(hi) - slope(lo).
    """
    assert bs % bpd == 0
    # Bacc (not raw Bass): its finalize() runs generate_event_semaphores,
    # which splits multi-sem waits (TRN2 allows 1 sync wait per instruction).
    nc = bacc.Bacc(None)
    x = nc.declare_dram_parameter("x", [bs, N, C], FP32, isOutput=False)
    ln_w = nc.declare_dram_parameter("ln_w", [C], FP32, isOutput=False)
    ln_b = nc.declare_dram_parameter("ln_b", [C], FP32, isOutput=False)
    W = nc.declare_dram_parameter("W", [1, C], FP32, isOutput=False)
    bias = nc.declare_dram_parameter("b", [1], FP32, isOutput=False)
    out = nc.declare_dram_parameter("out", [bs, 1], FP32, isOutput=True)

    with tile.TileContext(nc) as tc, ExitStack() as ctx:
        xpool = ctx.enter_context(tc.tile_pool(name="xp", bufs=x_bufs))
        singles = ctx.enter_context(tc.tile_pool(name="si", bufs=1))
        ep = ctx.enter_context(tc.tile_pool(name="ep", bufs=1))
        psum = ctx.enter_context(
            tc.tile_pool(name="ps", bufs=1, space=bass.MemorySpace.PSUM)
        )

        eps_t = singles.tile([P, 1], FP32)
        nc.vector.memset(eps_t, LN_EPS)

        # Prewarm the ACT function-set tables under the DMA stream so the
        # epilogue doesn't stall ~1.3us per table switch. Order matters:
        # Sigmoid first, Sqrt last, so the tail's Sqrt finds its set
        # resident and only the final Sigmoid pays one switch.
        prewarm = singles.tile([P, 1], FP32)
        nc.scalar.activation(
            out=prewarm,
            in_=eps_t,
            func=mybir.ActivationFunctionType.Sigmoid,
            bias=0.0,
            scale=1.0,
        )
        nc.scalar.activation(
            out=prewarm,
            in_=eps_t,
            func=mybir.ActivationFunctionType.Sqrt,
            bias=0.0,
            scale=1.0,
        )

        # Sliding-window one-hot bank: G[:, 128] = 1, else 0.
        # Stationary for batch b is G[:, P-b : 2P-b] (ones in column b).
        # float32r end-to-end: walrus's birverifier requires every tensor
        # consumed by an FP32r matmul to be produced as float32r; memset
        # can't emit fp32r, so memset fp32 then round via tensor_copy.
        G32 = singles.tile([P, 2 * P + 1], FP32)
        nc.vector.memset(G32, 0.0)
        nc.vector.memset(G32[:, P : P + 1], 1.0)
        G = singles.tile([P, 2 * P + 1], FP32R)
        nc.vector.tensor_copy(G, G32)

        def bcast_load(src_ap, ncols, name):
            """Replicate a [ncols] DRAM vector across all partitions."""
            t = singles.tile([P, ncols], FP32, name=name)
            bc = bass.AP(
                tensor=src_ap.tensor,
                offset=src_ap.offset,
                ap=[[0, P]] + [list(d) for d in src_ap.ap],
            )
            nc.gpsimd.dma_start(out=t, in_=bc)
            return t

        lnw_t = bcast_load(ln_w[:], C, "lnw_t")
        lnb_t = bcast_load(ln_b[:], C, "lnb_t")
        w_t = bcast_load(W[0], C, "w_t")
        b_t = bcast_load(bias[:], 1, "b_t")

        # Batch-independent epilogue precompute, hoisted before the main
        # loop so DVE does it under the DMA stream instead of in the tail:
        # wln = ln_w * W ; swln = sum(wln) ; c0 = sum(ln_b * W) + b
        # (DVE instructions encode at most ONE sync wait, so give each
        # broadcast-DMA'd tile a single-dependency first consumer.)
        wcopy = singles.tile([P, C], FP32)
        nc.vector.tensor_copy(wcopy, w_t)
        wln = singles.tile([P, C], FP32)
        nc.vector.tensor_mul(wln, lnw_t, wcopy)
        swln = ep.tile([P, 1], FP32)
        nc.vector.reduce_sum(out=swln, in_=wln, axis=mybir.AxisListType.X)
        # (tensor_tensor_reduce is avoided: its custom DVE ucode isn't
        # shipped via this compile path and it kills the exec unit.)
        scr0 = ep.tile([P, C], FP32)
        c0 = ep.tile([P, 1], FP32)
        nc.vector.tensor_mul(scr0, lnb_t, wcopy)
        nc.vector.reduce_sum(out=c0, in_=scr0, axis=mybir.AxisListType.X)
        nc.vector.tensor_add(c0, c0, b_t)

        # ---- main loop: psum[b, c] = sum_n x[b, n, c] on the PE ----
        acc_ps = psum.tile([P, C], FP32)  # 2 fp32 banks
        n_cch = C // MM_FD
        # Batch chunks per DMA: bpd-wide, except the final bpd batches go
        # as single-batch transfers so the tail's matmuls overlap the
        # second-to-last (smaller) transfer instead of trailing a full
        # bpd-wide one.
        chunks = [(b0, bpd) for b0 in range(0, max(bs - bpd, 0), bpd)]
        chunks += [(b0, 1) for b0 in range(max(bs - bpd, 0), bs)]
        rings = [nc.sync, nc.scalar][:DMA_RINGS]
        for _ in range(passes):
            for ti, (b0, w) in enumerate(chunks):
                # Node n = 2p + q: partition p holds consecutive node rows
                # {2p, 2p+1}, so each partition's DMA chunk is 8 KiB
                # contiguous (one descriptor) instead of 2x 4 KiB. The
                # matmul over q=0 and q=1 still sums all 256 nodes.
                xt = xpool.tile([P, w, NH, C], FP32R, name=f"xt{w}")
                src = x[b0 : b0 + w, :, :].rearrange(
                    "b (p q) c -> p b q c", p=P
                )
                rings[ti % len(rings)].dma_start(out=xt, in_=src.bitcast(FP32R))
                for j in range(w):
                    bidx = b0 + j
                    wap = G[:, P - bidx : 2 * P - bidx]
                    for h in range(NH):
                        first = bidx == 0 and h == 0
                        last = bidx == bs - 1 and h == NH - 1
                        for cc in range(n_cch):
                            nc.tensor.matmul(
                                acc_ps[:, cc * MM_FD : (cc + 1) * MM_FD],
                                wap,
                                xt[:, j, h, cc * MM_FD : (cc + 1) * MM_FD],
                                start=first,
                                stop=last,
                            )

        # ---- epilogue: all `bs` batches at once, partition = batch ----
        # s = acc_ps is read straight out of PSUM (DVE PSUM reads are 1x,
        # same as fp32 SBUF reads) — no staging copy in the tail. With
        # passes>1, only the final pass's values are read; intermediate
        # passes are pure stream (start=True resets PSUM each pass).
        s = acc_ps
        # dot = s . wln first: DVE does the elementwise product, then the
        # free-dim reduction runs on the SCALAR engine via activation
        # accum_out (a Copy whose outputs accumulate into [P,1]) — in
        # parallel with the DVE bn_stats chain below. Copy is in every
        # act-func set, so no table switch.
        scr1 = ep.tile([P, C], FP32)
        nc.vector.tensor_mul(scr1[:bs], s[:bs], wln[:bs])
        dot = ep.tile([P, 1], FP32)
        dscr = ep.tile([P, C], FP32)
        nc.scalar.activation(
            out=dscr[:bs],
            in_=scr1[:bs],
            func=mybir.ActivationFunctionType.Copy,
            bias=0.0,
            scale=1.0,
            accum_out=dot[:bs],
        )

        stats = ep.tile([P, 2, 6], FP32)
        sv = s.rearrange("p (g d) -> p g d", g=2)
        for g in range(2):
            nc.vector.bn_stats(out=stats[:bs, g, :], in_=sv[:bs, g, :])
        mv = ep.tile([P, 2], FP32)
        nc.vector.bn_aggr(out=mv[:bs], in_=stats[:bs])
        mu = mv[:bs, 0:1]
        var = mv[:bs, 1:2]

        std = ep.tile([P, 1], FP32)
        nc.scalar.activation(
            out=std[:bs],
            in_=var,
            func=mybir.ActivationFunctionType.Sqrt,
            bias=eps_t[:bs],
            scale=1.0,
        )
        # t0 has no dependency on sqrt — DVE computes it while ACT works.
        t0 = ep.tile([P, 1], FP32)
        nc.vector.tensor_mul(t0[:bs], mu, swln[:bs])
        rstd = ep.tile([P, 1], FP32)
        nc.vector.reciprocal(out=rstd[:bs], in_=std[:bs])

        # logits = rstd * (dot - mu * swln); out = sigmoid(logits + c0).
        # The rstd multiply folds into the sigmoid's per-partition scale.
        t1 = ep.tile([P, 1], FP32)
        nc.vector.tensor_sub(t1[:bs], dot[:bs], t0[:bs])
        res = ep.tile([P, 1], FP32)
        nc.scalar.activation(
            out=res[:bs],
            in_=t1[:bs],
            func=mybir.ActivationFunctionType.Sigmoid,
            bias=c0[:bs],
            scale=rstd[:bs],
        )
        nc.sync.dma_start(out=out[:, :], in_=res[:bs])

    # Run the Bacc compile pipeline (register allocation + multi-sync-wait
    # splitting via generate_event_semaphores) — nothing else in the
    # run_bass_kernel_spmd/axon path calls finalize for us.
    nc.finalize()
    return nc


_NC_CACHE = {}


def kernel(**inputs) -> np.ndarray:
    global LAST_RESULT
    x = np.ascontiguousarray(np.asarray(inputs["x"], dtype=np.float32))
    ln_w = np.ascontiguousarray(np.asarray(inputs["ln_w"], dtype=np.float32))
    ln_b = np.ascontiguousarray(np.asarray(inputs["ln_b"], dtype=np.float32))
    W = np.ascontiguousarray(np.asarray(inputs["W"], dtype=np.float32))
    b = np.ascontiguousarray(np.asarray(inputs["b"], dtype=np.float32))

    if "full" not in _NC_CACHE:
        _NC_CACHE["full"] = build()
    nc = _NC_CACHE["full"]

    in_maps = [
        {
            "x": x[i * BS : (i + 1) * BS],
            "ln_w": ln_w,
            "ln_b": ln_b,
            "W": W,
            "b": b,
        }
        for i in range(NCORES)
    ]
    res = run_bass_kernel_spmd(nc, in_maps, list(range(NCORES)))
    LAST_RESULT = res
    return np.concatenate([res.results[i]["out"] for i in range(NCORES)], axis=0)
